# revision 8
# baseline (speedup 1.0000x reference)
"""Trainium2 Bass kernel for DiceLoss (nn_DiceLoss_12326556140285).

Full (unsharded) contract: kernel(input, target, std) -> scalar np.ndarray.
Data-parallel over batch: 64 samples -> 8 cores x 8 samples. Inputs are
cast to bf16 on the host (halves HBM traffic; rel-err ~5e-4).

Math (per sample, z = (x - thr)/std, thr = 0.9*max(target)):
  s = sigmoid(z) = (1 + w)/2,  w = tanh(z/2)
  t = target > thr ;  H = x > thr ;  r = relu(w)
  x' = where(H == t, t, s)
  With f = x'|_{t=1} = (1 + H + (w - r))/2 and g = x'|_{t=0} = (r + H)/2:
    num = 2*sum(x' t) + 1e-5 = T + StH + Stw - Str + 1e-5
    den = sum(t) + sum(x') + 1e-5
        = 1.5*T + 0.5*Sr + 0.5*SH + 0.5*Stw - Str + 1e-5
    loss_b = 1 - num/den ;  output = mean_b loss_b

Engine split per core: DVE runs four 4x-mode bf16 tensor_scalar passes
(target max via op1=max accum, t-compare, H-compare, relu); ACT runs one
tanh pass; PE contracts t against the interleaved [w|r|H] blocks, with
PSUM diagonals (Stw, Str, StH) extracted by scalar_tensor_tensor
against an identity matrix.
"""

import numpy as np

N_CORES = 8
B = 64
SPC = B // N_CORES          # samples per core
FREE = 1024 * 1024 // 128   # 8192 free elems per partition per sample
N_ATOM = 8                  # T, SH, Sr, Stw, Str, StH, pad, pad

_COMPILED = {}


def build_nc(samples=SPC, free=FREE, n_chunks=2):
    import concourse.bass as bass
    import concourse.tile as tile
    from concourse import bacc, mybir, bass_isa

    f32 = mybir.dt.float32
    bf16 = mybir.dt.bfloat16
    Alu = mybir.AluOpType
    Act = mybir.ActivationFunctionType

    nt = free // 128          # matmul tiles per sample
    chunk = free // n_chunks  # DMA chunk (free elems)

    nc = bacc.Bacc("TRN2", target_bir_lowering=False, debug=False)
    inp_d = nc.dram_tensor("inp", [samples, 128, free], bf16, kind="ExternalInput").ap()
    tgt_d = nc.dram_tensor("tgt", [samples, 128, free], bf16, kind="ExternalInput").ap()
    std_d = nc.dram_tensor("std", [128, 1], f32, kind="ExternalInput").ap()
    eye_d = nc.dram_tensor("eye", [128, 128], f32, kind="ExternalInput").ap()
    out_d = nc.dram_tensor("out", [1, 1], f32, kind="ExternalOutput").ap()

    with tile.TileContext(nc) as tc:
        with (
            tc.tile_pool(name="const", bufs=1) as p_const,
            tc.tile_pool(name="tgt", bufs=2) as p_tgt,
            tc.tile_pool(name="x", bufs=2) as p_x,
            tc.tile_pool(name="tt", bufs=2) as p_t,
            tc.tile_pool(name="wah", bufs=2) as p_wah,
            tc.tile_pool(name="thr", bufs=2) as p_thr,
            tc.tile_pool(name="fin", bufs=16) as p_fin,
            tc.tile_pool(name="psum", bufs=2, space="PSUM") as p_psum,
        ):
            # ---- global constants ----
            eye = p_const.tile([128, 128], f32)
            nc.sync.dma_start(eye[:], eye_d[:])
            atoms = p_const.tile([128, samples * N_ATOM], f32)
            nc.vector.memset(atoms[:], 0.0)
            junk_f = p_const.tile([128, 128], f32)

            # 1/(2*std) and -1/(2*std) per partition (std replicated by host)
            std_sb = p_const.tile([128, 1], f32)
            nc.sync.dma_start(std_sb[:], std_d[:])
            std2 = p_const.tile([128, 1], f32)
            nc.vector.tensor_scalar_mul(std2[:], std_sb[:], 2.0)
            i2s = p_const.tile([128, 1], f32)
            nc.vector.reciprocal(i2s[:], std2[:])
            ni2s = p_const.tile([128, 1], f32)
            nc.vector.tensor_scalar_mul(ni2s[:], i2s[:], -1.0)

            for b in range(samples):
                ab = b * N_ATOM  # atom cols: T,SH,Sr,Stw,Str,StH

                # ---- target sample in SBUF; per-partition max via op1=max ----
                tgt_sb = p_tgt.tile([128, free], bf16)
                for c in range(n_chunks):
                    sl = slice(c * chunk, (c + 1) * chunk)
                    nc.sync.dma_start(tgt_sb[:, sl], tgt_d[b][:, sl])
                t_sb = p_t.tile([128, free], bf16)
                pmax = p_thr.tile([128, 1], f32)
                # out (scratch into t_sb, overwritten below) = max(tgt, 0) = tgt
                nc.vector.tensor_scalar(
                    t_sb[:], tgt_sb[:], 0.0, None, Alu.max, Alu.max,
                    accum_out=pmax[:],
                )
                allmax = p_thr.tile([128, 1], f32)
                nc.gpsimd.partition_all_reduce(
                    allmax[:], pmax[:], channels=128,
                    reduce_op=bass_isa.ReduceOp.max,
                )
                thr_t = p_thr.tile([128, 1], f32)
                nc.vector.tensor_scalar_mul(thr_t[:], allmax[:], 0.9)
                bias_t = p_thr.tile([128, 1], f32)  # -thr/(2 std)
                nc.vector.tensor_scalar(
                    bias_t[:], thr_t[:], ni2s[:], None, Alu.mult
                )

                # ---- input sample in SBUF ----
                x_sb = p_x.tile([128, free], bf16)
                for c in range(n_chunks):
                    sl = slice(c * chunk, (c + 1) * chunk)
                    nc.sync.dma_start(x_sb[:, sl], inp_d[b][:, sl])

                wah = p_wah.tile([128, 3 * free], bf16)
                wah4 = wah[:].rearrange("p (t k l) -> p t k l", t=nt, k=3, l=128)
                x_v = x_sb[:].rearrange("p (t l) -> p t l", l=128)
                w_v = wah4[:, :, 0, :]   # [128, nt, 128]
                r_v = wah4[:, :, 1, :]
                h_v = wah4[:, :, 2, :]

                # ---- ACT: w = tanh((x - thr)/(2 std)) ----
                nc.scalar.activation(
                    w_v, x_v, Act.Tanh,
                    bias=bias_t[:], scale=i2s[:],
                )

                # ---- DVE 4x passes: t (+T), H (+SH), relu(w) (+Sr) ----
                nc.vector.tensor_scalar(
                    t_sb[:], tgt_sb[:], thr_t[:], None, Alu.is_gt, Alu.add,
                    accum_out=atoms[:, ab + 0 : ab + 1],
                )
                nc.vector.tensor_scalar(
                    h_v, x_v, thr_t[:], None, Alu.is_gt, Alu.add,
                    accum_out=atoms[:, ab + 1 : ab + 2],
                )
                nc.vector.tensor_scalar(
                    r_v, w_v, 0.0, None, Alu.max, Alu.add,
                    accum_out=atoms[:, ab + 2 : ab + 3],
                )

                # ---- PE: psum[j1,j2] += sum_k t[k,j1] * [w|aw|H][k,j2] ----
                ps = p_psum.tile([128, 384], f32)
                for ti in range(nt):
                    nc.tensor.matmul(
                        ps[:],
                        t_sb[:, ti * 128 : (ti + 1) * 128],
                        wah[:, ti * 384 : (ti + 1) * 384],
                        start=(ti == 0),
                        stop=(ti == nt - 1),
                    )
                # diag extraction: Stw, Str, StH per-partition partials
                nc.vector.scalar_tensor_tensor(
                    junk_f[:], ps[:, 0:128], 1.0, eye[:],
                    Alu.mult, Alu.mult,
                    accum_out=atoms[:, ab + 3 : ab + 4],
                )
                nc.vector.scalar_tensor_tensor(
                    junk_f[:], ps[:, 128:256], 1.0, eye[:],
                    Alu.mult, Alu.mult,
                    accum_out=atoms[:, ab + 4 : ab + 5],
                )
                nc.vector.scalar_tensor_tensor(
                    junk_f[:], ps[:, 256:384], 1.0, eye[:],
                    Alu.mult, Alu.mult,
                    accum_out=atoms[:, ab + 5 : ab + 6],
                )

            # ---- final reduction & loss assembly ----
            allat = p_fin.tile([128, samples * N_ATOM], f32)
            nc.gpsimd.partition_all_reduce(
                allat[:], atoms[:], channels=128,
                reduce_op=bass_isa.ReduceOp.add,
            )
            a = allat[0:1, :].rearrange("p (b k) -> p b k", k=N_ATOM)
            T, SH, Sr, Stw, Str, StH = (a[:, :, j] for j in range(6))

            _tvn = [0]

            def tv():
                _tvn[0] += 1
                return p_fin.tile(
                    [1, samples], f32, tag="fintmp", name=f"fintmp{_tvn[0]}"
                )

            # num = T + StH + Stw - Str + 1e-5
            # den = 1.5*T + 0.5*Sr + 0.5*SH + 0.5*Stw - Str + 1e-5
            n1 = tv(); nc.vector.tensor_add(n1[:], T, StH)
            n2 = tv(); nc.vector.tensor_sub(n2[:], Stw, Str)
            n3 = tv(); nc.vector.tensor_add(n3[:], n1[:], n2[:])
            num = tv(); nc.vector.tensor_scalar_add(num[:], n3[:], 1e-5)

            d1 = tv(); nc.vector.tensor_add(d1[:], Sr, SH)
            d2 = tv(); nc.vector.tensor_add(d2[:], d1[:], Stw)
            t15 = tv(); nc.vector.tensor_scalar_mul(t15[:], T, 1.5)
            d3 = tv(); nc.vector.scalar_tensor_tensor(
                d3[:], d2[:], 0.5, t15[:], Alu.mult, Alu.add
            )
            d4 = tv(); nc.vector.tensor_sub(d4[:], d3[:], Str)
            den = tv(); nc.vector.tensor_scalar_add(den[:], d4[:], 1e-5)

            rv = tv(); nc.vector.reciprocal(rv[:], den[:])
            pv = tv(); nc.vector.tensor_mul(pv[:], num[:], rv[:])
            sv = p_fin.tile([1, 1], f32, tag="finsc")
            nc.vector.reduce_sum(out=sv[:], in_=pv[:], axis=mybir.AxisListType.X)
            # sum_b (1 - pv_b) / B  (partial over this core's samples)
            outsb = p_fin.tile([1, 1], f32, tag="finout")
            nc.vector.tensor_scalar(
                outsb[:], sv[:], -1.0 / B, float(samples) / B, Alu.mult, Alu.add
            )
            nc.sync.dma_start(out_d[:], outsb[:])

    nc.compile()
    return nc


def _get_compiled():
    if "nc" not in _COMPILED:
        _COMPILED["nc"] = build_nc()
    return _COMPILED["nc"]


def _make_in_maps(input, target, std):
    import ml_dtypes

    bf16 = ml_dtypes.bfloat16
    inp = np.asarray(input, dtype=np.float32).reshape(B, 128, FREE).astype(bf16)
    tgt = np.asarray(target, dtype=np.float32).reshape(B, 128, FREE).astype(bf16)
    stdv = np.full((128, 1), np.asarray(std, dtype=np.float32).reshape(-1)[0],
                   dtype=np.float32)
    eye = np.eye(128, dtype=np.float32)

    in_maps = []
    for c in range(N_CORES):
        sl = slice(c * SPC, (c + 1) * SPC)
        in_maps.append({
            "inp": np.ascontiguousarray(inp[sl]),
            "tgt": np.ascontiguousarray(tgt[sl]),
            "std": stdv,
            "eye": eye,
        })
    return in_maps


def kernel(input, target, std):
    from concourse.bass_utils import run_bass_kernel_spmd

    nc = _get_compiled()
    in_maps = _make_in_maps(input, target, std)
    res = run_bass_kernel_spmd(nc, in_maps, list(range(N_CORES)))
    total = np.float32(0.0)
    for c in range(N_CORES):
        total += np.float32(res.results[c]["out"][0, 0])
    return np.array(total, dtype=np.float32)
